# revision 56
# baseline (speedup 1.0000x reference)
"""Trainium2 Bass kernel for multi-scale conv QKV attention (nn_MGAFE).

Reference computation (B=1, H=W=64, C=256, NH=8 heads, c=32 per head):
  qkv = concat(conv1x1(x), conv3x3(x), conv5x5(x))   # q | k | v channel blocks
  qkv = depthwise3x3(qkv)
  q, k = l2norm(q), l2norm(k)  (per head, over the 32 chans)
  attn = softmax(q @ k^T * temperature); o = attn @ v
  out = conv1x1(o, Wproj)

Sharding, v3 (conv spatial -> attention head-sharded, overlapped AllToAlls):
 - conv phase: core c owns image rows [8c, 8c+8) = 512 positions; branch
   convs over a 10-row halo band in bf16, depthwise as diagonal matmuls,
   l2norm rsqrt via quake-Newton on the DVE (keeps the Act engine on ONE
   table set: Exp/Square/Copy only -> a single ACT_TABLE_LOAD).
 - branches run q (1x1), k (3x3), then v (5x5). The q+k AllToAll is issued
   before the v conv and hides under it; the v AllToAll hides under the
   attention qk/exp warmup (pv matmuls are issued LAG groups behind qk so
   the in-order PE queue never blocks on v's arrival).
 - attention (core h = head h, 4096 q x 4096 k): scoresT[m, q] =
   K-block^T @ Q so softmax sum and attn@v contract over m = partitions.
   exp on ScalarE straight from PSUM into bf16 (the ~110us/core exp stream
   is the phase's floor; qk/pv are interleaved beneath it). PV + ones-row
   denominator accumulate in PSUM; softmax reciprocal via DVE
   reciprocal_approx_fast + a PE ones-broadcast into the same PSUM bank.
 - a third small AllToAll returns o blocks [32, 512] to spatial bands;
   final 1x1 projection is local per 512-position band.
"""

import sys
import numpy as np

sys.path.insert(0, "/opt/trn_rl_repo")

import concourse.bass as bass  # noqa: E402
import concourse.tile as tile  # noqa: E402
from concourse import bacc, mybir  # noqa: E402

F32 = mybir.dt.float32
F32R = mybir.dt.float32r
BF16 = mybir.dt.bfloat16
I32 = mybir.dt.int32
FP8 = mybir.dt.float8e4
AF = mybir.ActivationFunctionType

NC = 8           # cores
H = W = 64
C = 256
NH = 8
HC = 32          # chans per head
ROWS = 8         # output rows per core
BR = ROWS + 2    # branch rows computed per core (dw halo)
XR = BR + 4      # x rows needed (5x5 halo on branch rows)
XCOL = W + 4     # padded x columns
BCOL = W + 2     # padded branch-buffer columns
QL = ROWS * W    # local positions per core (512)
N = H * W        # total positions (4096)
MB = N // 128    # 32 key blocks of 128

# attention mb-group sizes: 3+3 qk double-buffer banks + 2 pv banks = 8 PSUM
GRP = [(3 * i, 3) for i in range(10)] + [(30, 2)]
LAG = 24         # pv issue lag (groups) so qk/exp run ahead of v's arrival

QUAKE = 0x5F3759DF + 1   # rsqrt seed magic (+1 pre-folded for the xor-add form)
RECIP = 0x7EF477D5 + 1   # reciprocal seed magic (same folding)


def r32(ap):
    return ap.bitcast(F32R)


def _emit(tc, I, out_ap):
    nc = tc.nc

    # ---------------- persistent small constants ----------------
    const = tc.alloc_tile_pool(name="const", bufs=1)
    ident = const.tile([128, 128], BF16, tag="ident", name="ident")
    temp32 = const.tile([64, 1], F32, tag="temp32", name="temp32")   # temperature per (sub,qg,head)
    indT16 = const.tile([128, 64], BF16, tag="indT16", name="indT16")  # per-qgroup head indicators
    ind48 = const.tile([64, 8 * 128], BF16, tag="ind48", name="ind48")  # rsqrt broadcast selectors
    edge = const.tile([128, 2], F32, tag="edge", name="edge")        # top/bottom row validity
    ones32 = const.tile([1, 32], BF16, tag="ones32", name="ones32")  # denom broadcast lhsT
    wdw_sb = const.tile([128, 36], F32, tag="wdw", name="wdw")       # q+k octile depthwise weights

    # x band, channel-partition layout, padded: [2 ictiles][128, XR*XCOL] bf16
    xpool = tc.alloc_tile_pool(name="x", bufs=1)
    xb = []
    for t in range(2):
        xt = xpool.tile([128, XR * XCOL], BF16, tag=f"x{t}", name=f"x{t}")
        nc.sync.dma_start(xt[:], I["xb"][t])
        xb.append(xt)

    wt_specs = {  # branch -> (weight dram, ksize, pad)
        "q": ("w1t", 1, 0),
        "k": ("w3t", 3, 1),
        "v": ("w5t", 5, 2),
    }

    # prefetch all conv weights, ONE batched DMA per branch (SP queue);
    # the k+v depthwise diags ride between w_k and w_v
    wpool = tc.alloc_tile_pool(name="wts", bufs=1)
    wsb = {}
    wtiles = {}
    for branch in ("q", "k", "v"):
        wnm, ks, _ = wt_specs[branch]
        nst = ks * ks * 2
        wtiles[branch] = wpool.tile([128, nst * 256], BF16, tag=f"w_{branch}", name=f"w_{branch}")
        wsb[branch] = wtiles[branch].rearrange("p (s o) -> p s o", o=256)
    nc.sync.dma_start(wsb["q"], I["w1t"].rearrange("t i p o -> p (t i) o"))
    nc.sync.dma_start(wsb["k"], I["w3t"].rearrange("t i p o -> p (t i) o"))

    # depthwise diagonal matrices: q's 18 built on the (idle) DVE so the q
    # depthwise can start early; k+v's 36 arrive by DMA (issued below,
    # between w_k and w_v on the SP queue)
    nc.scalar.dma_start(wdw_sb[:], I["wdw"][:])
    nc.scalar.dma_start(ident[:], I["ident"][:])
    nc.scalar.dma_start(edge[:], I["edge"][:])
    nc.scalar.dma_start(temp32[:], I["temp32"][:])
    nc.scalar.dma_start(indT16[:], I["indT16"][:])
    nc.scalar.dma_start(ind48[:], I["ind48"][:])
    nc.scalar.dma_start(ones32[:], I["ones32"][:])
    diag_pool = tc.alloc_tile_pool(name="diag", bufs=1)
    diags = diag_pool.tile([128, 54 * 128], BF16, tag="diags", name="diags")
    dg3 = diags.rearrange("p (t c) -> p t c", c=128)
    for t in range(36):
        nc.vector.tensor_scalar_mul(dg3[:, t], ident[:], wdw_sb[:, t:t + 1])
    nc.sync.dma_start(dg3[:, 36:54], I["dwdiag"].rearrange("t p c -> p t c"))
    nc.sync.dma_start(wsb["v"], I["w5t"].rearrange("t i p o -> p (t i) o"))

    # AllToAll buffers (DRAM): q+k collective (sent early), v, o
    aqk_in = nc.dram_tensor("aqk_in", [NC, 2 * HC * QL], FP8)
    aqk_out = nc.dram_tensor("aqk_out", [NC, 2 * HC * QL], FP8)
    av_in = nc.dram_tensor("av_in", [NC, HC * QL], BF16)
    av_out = nc.dram_tensor("av_out", [NC, HC * QL], BF16)
    o_in = nc.dram_tensor("o_in", [NC, HC, QL], BF16)
    o_out = nc.dram_tensor("o_out", [NC, HC, QL], BF16)

    # chan-major [c, 512] views of the a2a qk blocks: q rows 0:32, k rows 32:64
    aqk_in_v = aqk_in[:].rearrange("h (c p) -> h c p", p=QL)
    aqk_out_v = aqk_out[:].rearrange("h (c p) -> h c p", p=QL)

    def conv_part(cpool, cps_pool, branch):
        wnm, ks, pad = wt_specs[branch]
        ntap = ks * ks
        # branch output buffers, [128, BR rows x BCOL cols], zero side pads
        bbufs, pss = [], []
        for sub in range(2):
            bbuf = cpool.tile([128, BR * BCOL], BF16, tag=f"bbuf{sub}", name=f"bbuf{sub}")
            b3 = bbuf.rearrange("p (r c) -> p r c", c=BCOL)
            nc.vector.memset(b3[:, :, 0:1], 0.0)          # left pad column
            nc.vector.memset(b3[:, :, BCOL - 1:BCOL], 0.0)  # right pad column
            bbufs.append(bbuf)
            pss.append([cps_pool.tile([128, 5 * W], F32, tag=f"cps{sub}{nb}", name=f"cps{sub}{nb}", bufs=1)
                        for nb in range(2)])
        steps = [(tap, t) for tap in range(ntap) for t in range(2)]
        for sub in range(2):
            for nb in range(2):
                for j, (tap, t) in enumerate(steps):
                    ky, kx = tap // ks, tap % ks
                    cs = kx + (2 - pad)
                    rs = nb * 5 + ky + (2 - pad)
                    x3 = xb[t].rearrange("p (r c) -> p r c", c=XCOL)
                    nc.tensor.matmul(
                        pss[sub][nb][:],
                        lhsT=wsb[branch][:, j, 128 * sub:128 * sub + 128],
                        rhs=x3[:, rs:rs + 5, cs:cs + W],
                        start=(j == 0), stop=(j == len(steps) - 1),
                        skip_group_check=True,
                    )
        for sub in range(2):
            bb3 = bbufs[sub].rearrange("p (r c) -> p r c", c=BCOL)
            for nb in range(2):
                nc.scalar.copy(bb3[:, nb * 5:nb * 5 + 5, 1:1 + W], pss[sub][nb][:])
            # zero out-of-image halo rows (top row on core 0, bottom on core 7)
            nc.vector.tensor_scalar_mul(bb3[:, 0, :], bb3[:, 0, :], edge[:, 0:1])
            nc.vector.tensor_scalar_mul(bb3[:, BR - 1, :], bb3[:, BR - 1, :], edge[:, 1:2])
        return bbufs

    def dw_part(dw_ps, bbufs, obase):
        # depthwise 3x3 as 9 diagonal matmuls per sub
        outs = []
        for sub in range(2):
            octile = obase + sub
            bb3 = bbufs[sub].rearrange("p (r c) -> p r c", c=BCOL)
            dps = dw_ps.tile([128, QL], F32, tag="dps", name="dps")
            for tap in range(9):
                ky, kx = tap // 3, tap % 3
                nc.tensor.matmul(
                    dps[:],
                    lhsT=dg3[:, octile * 9 + tap],
                    rhs=bb3[:, ky:ky + ROWS, kx:kx + W],
                    start=(tap == 0), stop=(tap == 8),
                    skip_group_check=True,
                )
            outs.append(dps)
        return outs

    def rsqrt_dve(lpool, dst, src_f32, scale):
        """dst[64, 128] f32 = 1/sqrt(src) via quake seed + 2 Newton steps (DVE only)."""
        it = lpool.tile([64, 128], I32, tag="rsq_i", name="rsq_i")
        nc.vector.tensor_scalar(it[:], src_f32.bitcast(I32), 1, None,
                                op0=mybir.AluOpType.logical_shift_right)
        # seed = QUAKE-1 - (i >> 1)  ==  (i>>1 XOR -1) + QUAKE
        # (bitwise and arith ops can't fuse in one tensor_scalar on HW)
        nc.vector.tensor_scalar(it[:], it[:], -1, None,
                                op0=mybir.AluOpType.bitwise_xor)
        nc.vector.tensor_scalar(it[:], it[:], QUAKE, None,
                                op0=mybir.AluOpType.add)
        y = it.bitcast(F32)
        a = lpool.tile([64, 128], F32, tag="rsq_a", name="rsq_a")
        nc.vector.tensor_mul(a[:], y[:], y[:])
        nc.vector.tensor_mul(a[:], a[:], src_f32)
        nc.vector.tensor_scalar(a[:], a[:], -0.5, 1.5,
                                op0=mybir.AluOpType.mult,
                                op1=mybir.AluOpType.add)
        # final Newton multiply fused with the temperature scale
        nc.vector.scalar_tensor_tensor(dst, y[:], scale, a[:],
                                       op0=mybir.AluOpType.mult,
                                       op1=mybir.AluOpType.mult)

    def l2norm_to_a2a(lpool, l2ps, evpool, dps2, off, with_temp):
        """l2-normalize both dps subs ([128, QL] PSUM each) in one fused pass
        and write the bf16 results into the a2a qk blocks of all 8 heads.
        The per-(head, query) sums are packed [48, 128] = (sub, qgroup, head)
        on partitions so the DVE Newton rsqrt costs free-size 128, not 512.
        High priority: this chain gates the q+k AllToAll."""
        sqs, dsbs = [], []
        for sub in range(2):
            sq = lpool.tile([128, QL], BF16, tag=f"sq{sub}", name=f"sq{sub}")
            nc.scalar.activation(sq[:], dps2[sub][:], AF.Square)
            sqs.append(sq)
            dsb = lpool.tile([128, QL], BF16, tag=f"dsb{sub}", name=f"dsb{sub}")
            nc.scalar.copy(dsb[:], dps2[sub][:])
            dsbs.append(dsb)
        ss = l2ps.tile([64, 128], F32, tag="ssq", name="ssq", bufs=1)
        nc.vector.memset(ss[:], 1.0)  # unused bands: keep Newton NaN-free
        for sub in range(2):
            for g in range(4):
                nc.tensor.matmul(ss[32 * sub:32 * sub + 16, :],
                                 lhsT=indT16[:, 16 * g:16 * g + 16],
                                 rhs=sqs[sub][:, 128 * g:128 * g + 128],
                                 start=(g == 0), stop=(g == 3),
                                 skip_group_check=True)
        rinv = lpool.tile([64, 128], BF16, tag="rinv", name="rinv")
        rsqrt_dve(lpool, rinv[:], ss[:],
                  temp32[:, 0:1] if with_temp else 1.0)
        for sub in range(2):
            rb = l2ps.tile([128, QL], F32, tag="rb", name="rb", bufs=1)
            for g in range(4):
                nc.tensor.matmul(rb[:, 128 * g:128 * g + 128],
                                 lhsT=ind48[:, 128 * (4 * sub + g):128 * (4 * sub + g) + 128],
                                 rhs=rinv[:],
                                 start=True, stop=True)
            hat = evpool.tile([128, QL], FP8, tag=f"hat{sub}", name=f"hat{sub}")
            nc.vector.tensor_mul(hat[:], dsbs[sub][:], rb[:])
            for hl in range(4):
                eng = nc.sync if hl % 2 == 0 else nc.scalar
                eng.dma_start(aqk_in_v[4 * sub + hl, off:off + HC],
                              hat[HC * hl:HC * hl + HC, :])

    evpool = tc.alloc_tile_pool(name="ev", bufs=2)
    cpool = tc.alloc_tile_pool(name="conv", bufs=2)
    lpool = tc.alloc_tile_pool(name="l2n", bufs=2)
    cps_pool = tc.alloc_tile_pool(name="cpsB", bufs=1, space="PSUM")
    dw_ps = tc.alloc_tile_pool(name="dwps", bufs=2, space="PSUM")
    l2ps = tc.alloc_tile_pool(name="l2ps", bufs=1, space="PSUM")

    # ---- q branch, then k branch (k's chain gates the q+k a2a), then a2a
    qbb = conv_part(cpool, cps_pool, "q")
    with tc.high_priority():
        qres = dw_part(dw_ps, qbb, 0)
        l2norm_to_a2a(lpool, l2ps, evpool, qres, 0, with_temp=True)
    kbb = conv_part(cpool, cps_pool, "k")
    with tc.high_priority():
        kres = dw_part(dw_ps, kbb, 2)
        l2norm_to_a2a(lpool, l2ps, evpool, kres, HC, with_temp=False)

    # q+k AllToAll: hides under the v conv below
    nc.gpsimd.collective_compute(
        "AllToAll", mybir.AluOpType.bypass,
        replica_groups=[list(range(NC))],
        ins=[aqk_in[:]], outs=[aqk_out[:]],
    )

    # ---- v branch (5x5) -> dw -> transpose -> a2a ([512, 32] pos-major)
    vbb = conv_part(cpool, cps_pool, "v")
    vres = dw_part(dw_ps, vbb, 4)
    vsb = []
    for sub in range(2):
        vs = evpool.tile([128, QL], BF16, tag=f"vsb{sub}", name=f"vsb{sub}")
        nc.vector.tensor_copy(vs[:], vres[sub][:])
        vsb.append(vs)
    l2ps.release()
    tpps = tc.alloc_tile_pool(name="tpps", bufs=2, space="PSUM")
    with tc.high_priority():
        for sub in range(2):
            # vtile free layout (head, pos-block, chan) so the send DMA folds
            vtile = evpool.tile([128, 4 * 128], BF16, tag=f"vt{sub}", name=f"vt{sub}")
            v4 = vtile.rearrange("p (hl b c) -> p hl b c", hl=4, b=4)
            for blk in range(4):
                tp = tpps.tile([128, 128], BF16, tag="tps", name="tps")
                nc.tensor.transpose(tp[:], vsb[sub][:, blk * 128:blk * 128 + 128], ident[:])
                nc.vector.tensor_copy(v4[:, :, blk, :], tp[:].rearrange("p (hl c) -> p hl c", c=HC))
            # one DMA for this sub's 4 heads: [512 pos, 32 ch] blocks
            dstv = av_in[4 * sub:4 * sub + 4].rearrange("h (b p c) -> p h b c", b=4, p=128)
            srcv = vtile.rearrange("p (hl b c) -> p hl b c", hl=4, b=4)
            nc.gpsimd.dma_start(dstv, srcv)

    # v AllToAll: hides under the attention qk/exp warmup (pv lags by LAG)
    nc.gpsimd.collective_compute(
        "AllToAll", mybir.AluOpType.bypass,
        replica_groups=[list(range(NC))],
        ins=[av_in[:]], outs=[av_out[:]],
    )

    for p in (tpps, dw_ps, cps_pool, lpool, cpool, evpool, diag_pool, wpool):
        p.release()

    # -------- attention (this core = head `core_id` over all 4096 pos) ------
    kv_pool = tc.alloc_tile_pool(name="kv", bufs=1)
    qh_sb = kv_pool.tile([32, N], FP8, tag="qh", name="qh")
    kh_sb = kv_pool.tile([32, N], FP8, tag="kh", name="kh")
    # progressive per-source loads: the first qk group only needs block 0
    for s in range(NC):
        nc.scalar.dma_start(kh_sb[:, QL * s:QL * s + QL], aqk_out_v[s, HC:2 * HC])
        nc.sync.dma_start(qh_sb[:, QL * s:QL * s + QL], aqk_out_v[s, 0:HC])
    # v with interleaved ones column: [128, mb x 33]
    v_all = kv_pool.tile([128, MB * 33], BF16, tag="vall", name="vall")
    nc.vector.memset(v_all[:], 1.0)
    v3m = v_all.rearrange("p (m x) -> p m x", x=33)
    for s in range(NC):
        srcv = av_out[s].rearrange("(b p c) -> p b c", b=4, p=128)
        nc.gpsimd.dma_start(v3m[:, 4 * s:4 * s + 4, 0:HC], srcv)

    qk_a = tc.alloc_tile_pool(name="qkA", bufs=1, space="PSUM")
    qk_b = tc.alloc_tile_pool(name="qkB", bufs=1, space="PSUM")
    acc_pool = tc.alloc_tile_pool(name="acc", bufs=2, space="PSUM")
    exp_pool = tc.alloc_tile_pool(name="expp", bufs=LAG + 2)
    fin_pool = tc.alloc_tile_pool(name="fin", bufs=2)

    # Issue order per step t: qk(t) on PE, pv(t-LAG) on PE, deferred fin,
    # exp(t) on Act. The LAG keeps every pv (which needs v_all from the v
    # AllToAll) out of the in-order PE queue until v has arrived, and lets
    # the Act engine run the exp stream back-to-back from the start.
    groups = [(qc, g0, gn) for qc in range(NC) for (g0, gn) in GRP]
    NG = len(groups)
    exs = {}
    pvt = {}
    fin_q = []  # [countdown, pv tile, qc]

    def issue_pv(s):
        qc, g0, gn = groups[s]
        if g0 == 0:
            pvt[qc] = acc_pool.tile([128, QL], F32, tag="pv", name="pv")
        ex = exs.pop(s)
        for j in range(gn):
            mb = g0 + j
            nc.tensor.matmul(
                pvt[qc][0:33, :],
                lhsT=v_all[:, 33 * mb:33 * mb + 33],
                rhs=ex[:, QL * j:QL * j + QL],
                start=(mb == 0), stop=(mb == MB - 1),
                skip_group_check=True,
            )
        if g0 + gn == MB:
            fin_q.append([2, pvt.pop(qc), qc])

    def issue_fin(pv_, oqc):
        # o = pv[0:32] * (1/pv[32]): quake-Newton reciprocal on the DVE
        # (basic ALU ops only -- reciprocal_approx_fast is sim-only), then
        # broadcast via a PE ones-matmul into rows 64-95 of the pv bank
        it = fin_pool.tile([1, QL], I32, tag="rz_i", name="rz_i")
        nc.vector.tensor_scalar(it[:], pv_[32:33, :].bitcast(I32), -1, None,
                                op0=mybir.AluOpType.bitwise_xor)
        nc.vector.tensor_scalar(it[:], it[:], RECIP, None,
                                op0=mybir.AluOpType.add)
        y = it.bitcast(F32)
        t = fin_pool.tile([1, QL], F32, tag="rz_t", name="rz_t")
        for _ in range(2):
            nc.vector.tensor_mul(t[:], y[:], pv_[32:33, :])
            nc.vector.tensor_scalar(t[:], t[:], -1.0, 2.0,
                                    op0=mybir.AluOpType.mult,
                                    op1=mybir.AluOpType.add)
            nc.vector.tensor_mul(y[:], y[:], t[:])
        rzb = fin_pool.tile([1, QL], BF16, tag="rzb", name="rzb")
        nc.vector.tensor_copy(rzb[:], y[:])
        nc.tensor.matmul(pv_[64:96, :], lhsT=ones32[:], rhs=rzb[:],
                         start=True, stop=True)
        rbs = fin_pool.tile([32, QL], F32, tag="rbs", name="rbs")
        nc.vector.tensor_copy(rbs[:], pv_[64:96, :])
        ob = fin_pool.tile([32, QL], BF16, tag="ob", name="ob")
        nc.vector.tensor_mul(ob[:], pv_[0:32, :], rbs[:])
        nc.sync.dma_start(o_in[oqc], ob[:])

    for t in range(NG + LAG):
        if t < NG:
            qc, g0, gn = groups[t]
            qk = (qk_a if t % 2 == 0 else qk_b).tile(
                [128, 3 * QL], F32, tag=f"qk{t % 2}", name=f"qk{t % 2}")
            for j in range(gn):
                mb = g0 + j
                nc.tensor.matmul(
                    qk[:, QL * j:QL * j + QL],
                    lhsT=kh_sb[:, 128 * mb:128 * mb + 128],
                    rhs=qh_sb[:, QL * qc:QL * qc + QL],
                    start=True, stop=True,
                )
        if t >= LAG:
            issue_pv(t - LAG)
        for f in fin_q:
            f[0] -= 1
        while fin_q and fin_q[0][0] <= 0:
            _, pv_, oqc = fin_q.pop(0)
            issue_fin(pv_, oqc)
        if t < NG:
            ex = exp_pool.tile([128, 3 * QL], BF16, tag="ex", name="ex")
            nc.scalar.activation(ex[:, 0:gn * QL], qk[:, 0:gn * QL], AF.Exp)
            exs[t] = ex
    while fin_q:
        _, pv_, oqc = fin_q.pop(0)
        issue_fin(pv_, oqc)

    nc.gpsimd.collective_compute(
        "AllToAll", mybir.AluOpType.bypass,
        replica_groups=[list(range(NC))],
        ins=[o_in[:]], outs=[o_out[:]],
    )

    for p in (fin_pool, exp_pool, acc_pool, qk_b, qk_a, kv_pool):
        p.release()

    # ---------------- projection ----------------
    wq = tc.alloc_tile_pool(name="wproj", bufs=1)
    wproj_sb = [wq.tile([128, C], BF16, tag=f"wp{t}", name=f"wp{t}") for t in range(2)]
    for t in range(2):
        nc.sync.dma_start(wproj_sb[t][:], I["wproj"][t])
    opool = tc.alloc_tile_pool(name="oall", bufs=1)
    o_sb = []
    for t in range(2):
        ot = opool.tile([128, QL], BF16, tag=f"osb{t}", name=f"osb{t}")
        eng = nc.sync if t == 0 else nc.scalar
        eng.dma_start(ot[:], o_out[4 * t:4 * t + 4].rearrange("s c p -> (s c) p"))
        o_sb.append(ot)
    pj_pool = tc.alloc_tile_pool(name="proj", bufs=1)
    pjps = tc.alloc_tile_pool(name="pjps", bufs=2, space="PSUM")
    oball = pj_pool.tile([128, 4 * C], F32, tag="oball", name="oball")
    ob4 = oball.rearrange("p (b o) -> p b o", o=C)
    for sblk in range(4):
        pp = pjps.tile([128, C], F32, tag="pj", name="pj")
        for t in range(2):
            nc.tensor.matmul(
                pp[:],
                lhsT=o_sb[t][:, 128 * sblk:128 * sblk + 128],
                rhs=wproj_sb[t][:],
                start=(t == 0), stop=(t == 1),
            )
        if sblk % 2 == 0:
            nc.scalar.copy(ob4[:, sblk], pp[:])
        else:
            nc.vector.tensor_copy(ob4[:, sblk], pp[:])
    nc.sync.dma_start(out_ap.rearrange("(b p) o -> p b o", b=4), ob4[:])

    for p in (pjps, pj_pool, opool, wq, xpool, const):
        p.release()


_CACHED = {}


def build_nc():
    if "nc" in _CACHED:
        return _CACHED["nc"]
    nc = bacc.Bacc("TRN2", target_bir_lowering=False, debug=False, num_devices=NC)
    shapes = {
        "xb": ([2, 128, XR * XCOL], BF16),
        "w1t": ([1, 2, 128, 256], BF16),
        "w3t": ([9, 2, 128, 256], BF16),
        "w5t": ([25, 2, 128, 256], BF16),
        "dwdiag": ([18, 128, 128], BF16),
        "wdw": ([128, 36], F32),
        "wproj": ([2, 128, C], BF16),
        "temp32": ([64, 1], F32),
        "indT16": ([128, 64], BF16),
        "ind48": ([64, 8 * 128], BF16),
        "ident": ([128, 128], BF16),
        "ones32": ([1, 32], BF16),
        "edge": ([128, 2], F32),
    }
    I = {n: nc.dram_tensor(n, s, dt, kind="ExternalInput").ap()
         for n, (s, dt) in shapes.items()}
    out_ap = nc.dram_tensor("out", [QL, C], F32, kind="ExternalOutput").ap()
    with tile.TileContext(nc) as tc:
        _emit(tc, I, out_ap)
    nc.compile()
    _CACHED["nc"] = nc
    return nc


def prep_inputs(x, W1, W3, W5, Wdw, Wproj, temperature):
    """Host-side sharding/layout prep. Returns per-core input dicts."""
    import ml_dtypes
    bf16 = ml_dtypes.bfloat16
    x = np.asarray(x, np.float32)
    W1 = np.asarray(W1, np.float32)
    W3 = np.asarray(W3, np.float32)
    W5 = np.asarray(W5, np.float32)
    Wdw = np.asarray(Wdw, np.float32)
    Wproj = np.asarray(Wproj, np.float32)
    temperature = np.asarray(temperature, np.float32)

    xc = np.transpose(x[0], (2, 0, 1))  # [C, H, W]
    xpad = np.zeros((C, H + 2 * 7, XCOL), np.float32)
    xpad[:, 7:7 + H, 2:2 + W] = xc  # extra top/bottom margin so bands slice cleanly

    def _temp48(temp_):
        t = np.ones((64, 1), np.float32)
        for s in range(2):
            for g in range(4):
                for h in range(4):
                    t[32 * s + 4 * g + h, 0] = temp_[4 * s + h, 0, 0]
        return t

    def _indT16():
        m = np.zeros((128, 64), np.float32)
        for g in range(4):
            for h in range(4):
                m[32 * h:32 * h + 32, 16 * g + 4 * g + h] = 1.0
        return m

    def _ind48():
        m = np.zeros((64, 8 * 128), np.float32)
        for s in range(2):
            for g in range(4):
                for p in range(128):
                    m[32 * s + 4 * g + p // 32, 128 * (4 * s + g) + p] = 1.0
        return m

    def _dwdiag(Wdw_, bft):
        wv = Wdw_.reshape(6, 128, 9)  # [octile, chan, tap]
        dd = np.zeros((18, 128, 128), np.float32)
        for oct_ in range(4, 6):
            for tap in range(9):
                np.fill_diagonal(dd[(oct_ - 4) * 9 + tap], wv[oct_, :, tap])
        return np.ascontiguousarray(dd).astype(bft)

    def wprep(Wt, ks):
        # [o, i, ky, kx] -> [tap, ictile, i, o]
        w = np.transpose(Wt, (2, 3, 1, 0)).reshape(ks * ks, 2, 128, 256)
        return np.ascontiguousarray(w).astype(bf16)

    ind4 = np.zeros((4, 128), np.float32)  # ind4[h, p] = 1 iff p // 32 == h
    for h in range(4):
        ind4[h, 32 * h:32 * h + 32] = 1.0

    shared = {
        "w1t": wprep(W1, 1),
        "w3t": wprep(W3, 3),
        "w5t": wprep(W5, 5),
        "dwdiag": _dwdiag(Wdw, bf16),
        "wdw": np.ascontiguousarray(
            Wdw.reshape(6, 128, 9)[0:4].transpose(1, 0, 2).reshape(128, 36),
            np.float32),
        "wproj": np.ascontiguousarray(
            np.transpose(Wproj[:, :, 0, 0], (1, 0)).reshape(2, 128, C)).astype(bf16),
        "temp32": _temp48(temperature),
        "indT16": _indT16().astype(bf16),
        "ind48": _ind48().astype(bf16),
        "ident": np.eye(128, dtype=np.float32).astype(bf16),
        "ones32": np.ones((1, 32), np.float32).astype(bf16),
    }

    in_maps = []
    for c in range(NC):
        r0 = 8 * c - 3  # first x row of the band (image coords)
        band = xpad[:, r0 + 7:r0 + 7 + XR, :]  # [C, XR, XCOL]
        xbc = band.reshape(2, 128, XR * XCOL).astype(bf16)
        edge = np.ones((128, 2), np.float32)
        if c == 0:
            edge[:, 0] = 0.0
        if c == NC - 1:
            edge[:, 1] = 0.0
        m = dict(shared)
        m["xb"] = np.ascontiguousarray(xbc)
        m["edge"] = edge
        in_maps.append(m)
    return in_maps


def run(inputs, trace=False):
    from concourse.bass_utils import run_bass_kernel_spmd
    nc = build_nc()
    in_maps = prep_inputs(**inputs)
    res = run_bass_kernel_spmd(nc, in_maps, list(range(NC)), trace=trace)
    bands = [res.results[c]["out"] for c in range(NC)]
    full = np.concatenate(bands, axis=0).reshape(1, H, W, C)
    return full, res


def kernel(**inputs):
    full, _ = run(inputs, trace=False)
    return full
